# revision 2
# baseline (speedup 1.0000x reference)
"""Multi-head attention (N=4, T=2048, D=512, H=8, dh=64) on 8 TRN2 NeuronCores.

Sharding: batch N (4) x head-group (2 groups of 4 heads) -> 8 cores.

v6: v5's attention core (64x128 2-tile score/AV matmuls with fused
denominator row) + pipeline restructure:
  - scratch warmup matmuls run during DMA staging so the PE HAM clock
    is at 8/8 when projections start;
  - K-projection is emitted first, then Q for the first q-block; the
    first super-pair's score rounds start immediately after, and the
    remaining Q/V projections are injected as backlog quanta into the
    first super-pair's rounds (projection fully overlaps attention);
  - exp rounds split 8/8 between ScalarE (native Exp) and VectorE
    (Schraudolph tensor_scalar) so neither engine paces the pipeline;
  - staging DMAs are fewer/fatter ([128, KS, T] packed layout) and
    ordered wk, k, wq, q-tb0, wv, rest so K lands first.
Per k-tile round, the head-lo score matmul (T0 = SBUF partitions 0-63)
and head-hi (T8 = 64-127) land in the SAME [128, 1024] PSUM ring slot
-> concurrent. One exp op per round covers both heads via the
interleaved pt layout [128, kt, head, 512]. AV splits each K=128
contraction into T0/T8 halves accumulating in four exclusive PSUM
banks; partials are copied out as their sweep finishes and summed on
the HOST. Row 64 of each [65, 512] block is the softmax denominator;
the HOST divides+transposes.
"""

import math

import ml_dtypes
import numpy as np

import concourse.bass as bass
import concourse.mybir as mybir
import concourse.tile as tile
from concourse import bacc
from concourse.bass_utils import run_bass_kernel_spmd

F32 = mybir.dt.float32
BF16 = mybir.dt.bfloat16
I16 = mybir.dt.int16
EXP = mybir.ActivationFunctionType.Exp
MULT = mybir.AluOpType.mult
ADD = mybir.AluOpType.add

N, T, D = 4, 2048, 512
HPC, DH = 4, 64
GC = HPC * DH
SCALE = 1.0 / math.sqrt(D)
QB = 512
NQB = T // QB            # 4
NKT = T // 128           # 16
KS = D // 128            # 4
OROW = DH + 1            # 65
ORWS = HPC * OROW        # 260

DVE_KTS = frozenset((0, 2, 4, 6, 8, 10, 12, 14))   # rounds exp'd by VectorE
SCH_C = 4.0
SCH_A = (128.0 / math.log(2.0)) * SCALE
SCH_B = 127.0 * 128.0 - SCH_C

N_WARM = 18


def build():
    nc = bacc.Bacc("TRN2", target_bir_lowering=False, debug=False, num_devices=8)
    qT_in = nc.declare_dram_parameter("qT", [128, KS * T], BF16, isOutput=False)
    kT_in = nc.declare_dram_parameter("kT", [128, KS * T], BF16, isOutput=False)
    wq_in = nc.declare_dram_parameter("wq", [128, KS * GC], BF16, isOutput=False)
    wk_in = nc.declare_dram_parameter("wk", [128, KS * GC], BF16, isOutput=False)
    wv_in = nc.declare_dram_parameter("wv", [128, KS * GC], BF16, isOutput=False)
    oT_out = nc.declare_dram_parameter("oT65", [2 * ORWS, T], F32, isOutput=True)

    with tile.TileContext(nc) as tc:
        with (
            tc.tile_pool(name="stage", bufs=2) as stage,
            tc.tile_pool(name="const", bufs=1) as const,
            tc.tile_pool(name="act", bufs=1) as actp,
            tc.tile_pool(name="ptc", bufs=3) as ptcp,
            tc.tile_pool(name="ost", bufs=8) as ostp,
            tc.tile_pool(name="ring", bufs=3, space="PSUM") as ring,  # 6 banks
            tc.tile_pool(name="psO", bufs=2, space="PSUM") as psO,    # 2 banks
        ):
            # ---- PE warmup on scratch (keeps HAM at 8/8 through staging) ----
            scratch = const.tile([128, 640], BF16, tag="scr")
            nc.gpsimd.memset(scratch[:], 0.0)
            for w in range(N_WARM):
                wp = psO.tile([128, 512], F32, tag="O", name=f"warm{w}")
                nc.tensor.matmul(wp[:], scratch[:, 0:128], scratch[:, 128:640],
                                 start=True, stop=True)

            # ---- input staging: packed [128, KS, T], K first ----
            kin = stage.tile([128, KS, T], BF16, tag="kin")
            qin = stage.tile([128, KS, T], BF16, tag="qin")
            wv = const.tile([128, KS, GC], BF16, tag="wv")
            wk = const.tile([128, KS, GC], BF16, tag="wk")
            wq = const.tile([128, KS, GC], BF16, tag="wq")

            kT_r = kT_in.rearrange("p (s t) -> p s t", s=KS)
            qT_r = qT_in.rearrange("p (s t) -> p s t", s=KS)
            nc.sync.dma_start(wk[:], wk_in.rearrange("p (s c) -> p s c", s=KS))
            nc.sync.dma_start(kin[:, :, 0:QB], kT_r[:, :, 0:QB])
            nc.sync.dma_start(wq[:], wq_in.rearrange("p (s c) -> p s c", s=KS))
            nc.sync.dma_start(qin[:, :, 0:QB], qT_r[:, :, 0:QB])
            for tb in range(1, NQB):
                nc.sync.dma_start(
                    kin[:, :, tb * QB : (tb + 1) * QB],
                    kT_r[:, :, tb * QB : (tb + 1) * QB])
            nc.sync.dma_start(wv[:], wv_in.rearrange("p (s c) -> p s c", s=KS))
            for tb in range(1, NQB):
                nc.sync.dma_start(
                    qin[:, :, tb * QB : (tb + 1) * QB],
                    qT_r[:, :, tb * QB : (tb + 1) * QB])

            kT_att = [actp.tile([128, T], BF16, tag=f"ka{d}", name=f"ka{d}")
                      for d in range(2)]
            qT_att = [actp.tile([128, T], BF16, tag=f"qa{d}", name=f"qa{d}")
                      for d in range(2)]

            vp = const.tile([128, NKT, HPC, OROW], BF16, tag="vp")
            ones_f32 = const.tile([128, NKT * HPC], F32, tag="ones")
            nc.gpsimd.memset(ones_f32[:], 1.0)
            nc.vector.tensor_copy(
                vp[:, :, :, DH : DH + 1],
                ones_f32[:].rearrange("p (a b) -> p a b", b=HPC).unsqueeze(3))

            # ---- projections: full-array K=128 ----
            def emit_kqproj(which, tb, dt2):
                w, src, dst = (
                    (wk, kin, kT_att) if which == "k" else (wq, qin, qT_att))
                cols = slice(tb * QB, (tb + 1) * QB)
                ps = psO.tile([128, QB], F32, tag="O",
                              name=f"{which}p{tb}_{dt2}")
                for s in range(KS):
                    nc.tensor.matmul(
                        ps[:], w[:, s, dt2 * 128 : (dt2 + 1) * 128],
                        src[:, s, cols], start=(s == 0), stop=(s == KS - 1))
                nc.vector.tensor_copy(dst[dt2][:, cols], ps[:])

            def emit_vproj(tt):
                ps = psO.tile([128, QB], F32, tag="O", name=f"vp{tt}")
                for s in range(KS):
                    nc.tensor.matmul(
                        ps[:, 0:GC], kin[:, s, tt * 128 : (tt + 1) * 128],
                        wv[:, s, :], start=(s == 0), stop=(s == KS - 1))
                nc.vector.tensor_copy(
                    vp[:, tt, :, 0:DH],
                    ps[:, 0:GC].rearrange("p (h d) -> p h d", d=DH))

            # serial pre-attention projections: all K, then Q for tb0
            for tb in range(NQB):
                for dt2 in range(2):
                    emit_kqproj("k", tb, dt2)
            for dt2 in range(2):
                emit_kqproj("q", 0, dt2)

            # ---- attention: pure 64x128 2-tile mode ----
            # pt layout: [128, kt, head(2), 512] bf16
            def emit_out(t2, qb, po, which, pi):
                hp = 2 * t2 + (0 if which == "lo" else 1)
                st = ostp.tile([128, QB], F32, tag="ost", name=f"o{which}{pi}")
                if (pi + (0 if which == "lo" else 1)) % 2 == 0:
                    nc.scalar.copy(st[0:OROW, :], po[0:OROW, :])
                else:
                    nc.vector.tensor_copy(st[0:OROW, :], po[0:OROW, :])
                nc.sync.dma_start(
                    oT_out[pi * ORWS + hp * OROW : pi * ORWS + (hp + 1) * OROW,
                           qb * QB : (qb + 1) * QB],
                    st[0:OROW, :])

            def build_av_quanta(t2, qb, pt):
                """AV work for one super-pair as a list of closures, to be
                interleaved into the NEXT super-pair's score rounds. PSUM
                banks are allocated lazily inside the closures so sweep 2
                reuses sweep 1's two banks after their evacuation (2-bank
                AV footprint; pop order makes the handoff safe)."""
                hp_lo, hp_hi = 2 * t2, 2 * t2 + 1
                hold = {}

                def pair1(kt):
                    def go():
                        if "lo0" not in hold:
                            hold["lo0"] = psO.tile([128, QB], F32, tag="O",
                                                   name="po_lo0")
                            hold["hi1"] = psO.tile([128, QB], F32, tag="O",
                                                   name="po_hi1")
                        nc.tensor.matmul(
                            hold["lo0"][0:OROW], vp[0:64, kt, hp_lo, :],
                            pt[0:64, kt, 0, :],
                            start=(kt == 0), stop=(kt == NKT - 1))
                        nc.tensor.matmul(
                            hold["hi1"][0:OROW], vp[64:128, kt, hp_hi, :],
                            pt[64:128, kt, 1, :],
                            start=(kt == 0), stop=(kt == NKT - 1))
                    return go

                def pair2(kt):
                    def go():
                        if "hi0" not in hold:
                            hold["hi0"] = psO.tile([128, QB], F32, tag="O",
                                                   name="po_hi0")
                            hold["lo1"] = psO.tile([128, QB], F32, tag="O",
                                                   name="po_lo1")
                        nc.tensor.matmul(
                            hold["hi0"][0:OROW], vp[0:64, kt, hp_hi, :],
                            pt[0:64, kt, 1, :],
                            start=(kt == 0), stop=(kt == NKT - 1))
                        nc.tensor.matmul(
                            hold["lo1"][0:OROW], vp[64:128, kt, hp_lo, :],
                            pt[64:128, kt, 0, :],
                            start=(kt == 0), stop=(kt == NKT - 1))
                    return go

                quanta = []
                for kt in range(NKT):
                    quanta.append(pair1(kt))
                quanta.append(lambda: emit_out(t2, qb, hold["lo0"], "lo", 0))
                quanta.append(lambda: emit_out(t2, qb, hold["hi1"], "hi", 1))
                for kt in range(NKT):
                    quanta.append(pair2(kt))
                quanta.append(lambda: emit_out(t2, qb, hold["hi0"], "hi", 0))
                quanta.append(lambda: emit_out(t2, qb, hold["lo1"], "lo", 1))
                return quanta

            def emit_scores(t2, qb, pt, backlog):
                q_lo = qT_att[t2][0:DH, qb * QB : (qb + 1) * QB]
                q_hi = qT_att[t2][DH:128, qb * QB : (qb + 1) * QB]
                for kt in range(NKT):
                    # inject prev super-pair's AV work ahead of the round's
                    # (possibly ring-blocked) score matmuls
                    nq = -(-len(backlog) // (NKT - kt))  # ceil
                    for _ in range(nq):
                        backlog.pop(0)()
                    sl = ring.tile([128, 2 * QB], F32, tag="R", name=f"sc{kt}")
                    nc.tensor.matmul(
                        sl[:, 0:QB],
                        kT_att[t2][0:DH, kt * 128 : (kt + 1) * 128],
                        q_lo, start=True, stop=True)
                    nc.tensor.matmul(
                        sl[:, QB : 2 * QB],
                        kT_att[t2][DH:128, kt * 128 : (kt + 1) * 128],
                        q_hi, start=True, stop=True)
                    if kt in DVE_KTS:
                        nc.vector.tensor_scalar(
                            pt[:, kt, :, :].bitcast(I16), sl[:],
                            SCH_A, SCH_B, MULT, ADD)
                    else:
                        nc.scalar.activation(pt[:, kt, :, :], sl[:], EXP,
                                             scale=SCALE)

            sps = [(qb, t2) for qb in range(NQB) for t2 in range(2)]
            # remaining projections overlap the first super-pair's rounds
            backlog = [
                (lambda tb_, d_: lambda: emit_kqproj("q", tb_, d_))(tb, d)
                for tb in range(1, NQB) for d in range(2)
            ] + [(lambda tt_: lambda: emit_vproj(tt_))(tt) for tt in range(NKT)]
            for i, (qb, t2) in enumerate(sps):
                pt = ptcp.tile([128, NKT, 2, QB], BF16, tag="ptc", name="ptc")
                emit_scores(t2, qb, pt, backlog)
                assert not backlog
                backlog = build_av_quanta(t2, qb, pt)
            for q_ in backlog:
                q_()

    nc.compile()
    return nc


_NC = None


def _get_nc():
    global _NC
    if _NC is None:
        _NC = build()
    return _NC


def _prep_w(W, cols):
    w = W[:, cols].astype(ml_dtypes.bfloat16)           # [512, 256]
    w = w.reshape(KS, 128, GC).transpose(1, 0, 2)       # [128, KS, GC]
    return np.ascontiguousarray(w.reshape(128, KS * GC))


def _prep_x(x):
    # [T, 512] -> [128, KS*T] with xp[p, s*T + t] = x[t, s*128 + p]
    xt = x.T.astype(ml_dtypes.bfloat16)                 # [512, T]
    xt = xt.reshape(KS, 128, T).transpose(1, 0, 2)      # [128, KS, T]
    return np.ascontiguousarray(xt.reshape(128, KS * T))


def run(query, key, W_query, W_key, W_value, trace=False):
    nc = _get_nc()
    query = np.asarray(query, dtype=np.float32)
    key = np.asarray(key, dtype=np.float32)
    W_query = np.asarray(W_query, dtype=np.float32)
    W_key = np.asarray(W_key, dtype=np.float32)
    W_value = np.asarray(W_value, dtype=np.float32)

    in_maps = []
    for c in range(8):
        n, g = c // 2, c % 2
        cols = slice(g * GC, (g + 1) * GC)
        in_maps.append(
            {
                "qT": _prep_x(query[n]),
                "kT": _prep_x(key[n]),
                "wq": _prep_w(W_query, cols),
                "wk": _prep_w(W_key, cols),
                "wv": _prep_w(W_value, cols),
            }
        )
    res = run_bass_kernel_spmd(nc, in_maps, core_ids=list(range(8)), trace=trace)
    out = np.empty((N, T, D), dtype=np.float32)
    for c in range(8):
        n, g = c // 2, c % 2
        r = res.results[c]["oT65"]  # [520, 2048]
        full = r[0:ORWS] + r[ORWS : 2 * ORWS]
        for hp in range(HPC):
            blk = full[hp * OROW : (hp + 1) * OROW]
            out[n, :, g * GC + hp * DH : g * GC + (hp + 1) * DH] = (
                blk[0:DH] / blk[DH : DH + 1]
            ).T
    return out, res


def kernel(query, key, W_query, W_key, W_value):
    out, _ = run(query, key, W_query, W_key, W_value, trace=False)
    return out
